# revision 1
# baseline (speedup 1.0000x reference)
"""Trainium2 Bass kernel for BatchAll triplet loss.

Reference computation (B=512, D=1024):
    pw = img @ sent.T                                  [B, B]
    t[a,p,n] = pw[a,p] - pw[a,n] + margin
    valid[a,p,n] = (lab[a]==lab[p]) & (lab[a]!=lab[n])
    loss = sum(relu(valid*t)) / (count(valid*t > EPS) + EPS)

Strategy: the batch is class-sorted on the host (a pure permutation of the
(image, sentence, label) triples; the loss is permutation invariant), then
anchors are sharded across 8 cores (64 each, C = core*64). After sorting,
the positives of anchor g all live in a contiguous class run within
(g-16, g+16) as long as every class has <= 16 members (checked on host;
dense fallback otherwise). So for anchor a (local), the p-axis can be
restricted to a 64-wide, 32-aligned window inside the core's 128-wide
sentence window [C-32, C+96).

Per core:
    pw rows over full n [64, 512] and over the window [64, 128] (PE).
    w[a,pwin] = pw+margin if same label else -1e30      [64, 128]
    z[a,n]    = -pw       if label differs else -1e30   [64, 512] bf16
    wT = transpose(w)  -> per-anchor bias columns       [128, 64]
    Main loop packs TWO anchors per tile: partitions = 2 x 64-window,
    free = all 512 n.  PE broadcasts the two z rows via a two-hot
    selector matmul; ACT applies relu(z + w) with the stacked window
    bias and accum_out row-sums; DVE counts r > EPS with accum_out.
Host combines the 8 (sum, count) pairs and divides.
"""

import numpy as np
from contextlib import ExitStack

B = 512
D = 1024
NCORES = 8
A = B // NCORES   # 64 anchors per core
KT = D // 128     # 8 contraction tiles
NT = B // 128     # 4 n-tiles per anchor (dense variant)
W = 128           # per-core sentence window width
MARGIN = 0.2
EPS = 1e-16
BIG = 1e30
MAXC_WIN = 21     # windowed variant valid iff max class size <= this

_CACHE = {}


def _lo_local(a):
    """32-aligned offset of anchor a's 64-wide window inside the core's
    128-wide window (anchor a sits at local window position 32 + a)."""
    return 32 * ((a - 15) // 32) + 32


def _build_win():
    """Class-sorted windowed kernel (primary)."""
    import concourse.bass as bass
    import concourse.mybir as mybir
    import concourse.tile as tile
    from concourse import bacc
    from concourse.masks import make_identity

    f32 = mybir.dt.float32
    bf16 = mybir.dt.float16
    BIGW = 30000.0
    Alu = mybir.AluOpType
    Act = mybir.ActivationFunctionType
    Ax = mybir.AxisListType

    nc = bacc.Bacc("TRN2", target_bir_lowering=False, debug=False,
                   num_devices=NCORES)

    NG = 21  # groups of 3 anchors (anchor 63 = tail)
    packT_d = nc.dram_tensor("packT", [D, A + B], bf16, kind="ExternalInput")
    labf_d = nc.dram_tensor("labf", [B], bf16, kind="ExternalInput")
    labc_d = nc.dram_tensor("labc", [A], f32, kind="ExternalInput")
    out_d = nc.dram_tensor("out", [2], f32, kind="ExternalOutput")
    wdram_d = nc.dram_tensor("wdram", [A, W], bf16)

    with tile.TileContext(nc) as tc:
        with ExitStack() as ctx:
            singles = ctx.enter_context(tc.tile_pool(name="singles", bufs=1))
            rpool = ctx.enter_context(tc.tile_pool(name="rpool", bufs=6))
            mpool = ctx.enter_context(tc.tile_pool(name="mpool", bufs=6))
            spsum = ctx.enter_context(
                tc.tile_pool(name="spsum", bufs=1, space="PSUM"))
            wpsum = ctx.enter_context(
                tc.tile_pool(name="wpsum", bufs=3, space="PSUM"))

            # ---- constants ----
            ones_r = singles.tile([1, A], bf16)
            nc.vector.memset(ones_r, 1.0)
            ones_c = singles.tile([128, 1], f32)
            nc.vector.memset(ones_c, 1.0)
            # selbias[k, ar, h, v] = 1 iff k == ar + 21*h: each matmul tile
            # stacks THREE anchors (ar, ar+21, ar+42), each with a 42-wide
            # window. Row 64 is filled later with the window bias w.
            # Anchor 63 is handled by a separate small tail tile.
            selbias = singles.tile([A + 1, NG, 3, 42], bf16)
            nc.gpsimd.memset(selbias, 0.0)
            nc.gpsimd.affine_select(
                out=selbias, in_=selbias, compare_op=mybir.AluOpType.not_equal,
                fill=1.0, base=0, pattern=[[-1, NG], [-21, 3], [0, 42]],
                channel_multiplier=1)
            selbias_t = singles.tile([A + 1, 42], bf16)
            nc.gpsimd.memset(selbias_t, 0.0)
            nc.gpsimd.affine_select(
                out=selbias_t, in_=selbias_t,
                compare_op=mybir.AluOpType.not_equal, fill=1.0, base=-63,
                pattern=[[0, 42]], channel_multiplier=1)

            # ---- loads: labels first (they gate the DVE mask chain),
            # packT split per k-tile over two queues so PE starts early ----
            lab_row = singles.tile([1, B], bf16)
            nc.scalar.dma_start(
                out=lab_row, in_=labf_d.ap().rearrange("(o b) -> o b", o=1))
            labc_col = singles.tile([A, 1], f32)
            nc.scalar.dma_start(
                out=labc_col, in_=labc_d.ap().rearrange("(a o) -> a o", o=1))
            packT = singles.tile([128, KT, A + B], bf16)
            packT_v = packT_d.ap().rearrange("(t p) m -> p t m", p=128)
            for kt in range(KT):
                eng = nc.sync if kt % 2 == 0 else nc.scalar
                eng.dma_start(out=packT[:, kt, :], in_=packT_v[:, kt, :])
            imgT = packT[:, :, 0:A]
            sentT = packT[:, :, A:A + B]

            # ---- label broadcast + masks (first PE matmul; DVE runs the
            # mask chain while the pairwise matmuls stream in) ----
            labB_ps = spsum.tile([A, B], f32, tag="sB")
            nc.tensor.matmul(labB_ps, lhsT=ones_r, rhs=lab_row)
            eqP = singles.tile([A, B], f32)
            nc.vector.tensor_scalar(eqP, labB_ps, labc_col, None, Alu.is_equal)
            eqW = eqP[:, 0:W]
            penW = singles.tile([A, W], f32)
            nc.vector.tensor_scalar(penW, eqW, 1.0, BIGW, Alu.subtract, Alu.mult)
            negneq = singles.tile([A, B], f32)
            nc.vector.tensor_scalar(negneq, eqP, 1.0, -1.0, Alu.subtract,
                                    Alu.mult)

            # ---- pairwise rows (sentT pre-rotated: window = cols 0..W) ----
            pw_ps = spsum.tile([A, B], f32, tag="sA")
            for kt in range(KT):
                nc.tensor.matmul(pw_ps, lhsT=imgT[:, kt, :], rhs=sentT[:, kt, :],
                                 start=(kt == 0), stop=(kt == KT - 1))

            # ---- w rows first (the selbias DMAs gate the main loop):
            # w = (pw+margin)*eqW + (eqW-1)*BIGW over the window ----
            w_win = singles.tile([A, W], f32)
            nc.vector.scalar_tensor_tensor(w_win, pw_ps[:, 0:W], MARGIN, eqW,
                                           Alu.add, Alu.mult)
            w_bf = singles.tile([A, W], bf16)
            nc.vector.tensor_add(w_bf, w_win, penW)
            # fold w into the selector: selbias row 64 carries the window
            # bias, matched by the ones row 64 of zext, so each matmul
            # emits t = z + w directly into PSUM. Anchor a's 42-wide window
            # starts at local column 11+a (diagonal), so bounce w through
            # DRAM and gather with an affine AP.
            nc.sync.dma_start(out=wdram_d.ap(), in_=w_bf)
            diag = bass.AP(tensor=wdram_d, offset=11,
                           ap=[[0, 1], [129, NG], [21 * 129, 3], [1, 42]])
            nc.sync.dma_start(out=selbias[64:65, :, :, :], in_=diag)
            tail = bass.AP(tensor=wdram_d, offset=63 * 129 + 11,
                           ap=[[0, 1], [1, 42]])
            nc.gpsimd.dma_start(out=selbias_t[64:65, :], in_=tail)

            # ---- z rows: zext[0:A] = -pw*(1-eqP) - BIGW*eqP ----
            z1 = singles.tile([A, B], f32)
            nc.vector.scalar_tensor_tensor(z1, pw_ps, -1.0, negneq,
                                           Alu.mult, Alu.mult)
            zext = singles.tile([A + 1, B], bf16)
            nc.vector.scalar_tensor_tensor(zext[0:A, :], eqP, -BIGW, z1,
                                           Alu.mult, Alu.add)
            nc.vector.memset(zext[A:A + 1, :], 1.0)

            # ---- accumulators ----
            NCOL = 12  # 10 dual-group + 1 single-group + 1 tail column
            Sacc = singles.tile([128, NCOL], f32)
            Cacc = singles.tile([128, NCOL], f32)
            nc.vector.memset(Sacc, 0.0)
            nc.vector.memset(Cacc, 0.0)
            MP = 3 * 42  # 126 partitions per group tile

            # ---- main loop: 6 anchors / 2 PSUM banks per iteration ----
            for i in range(10):
                zb_ps = wpsum.tile([128, 2, B], f32)
                for u in range(2):
                    nc.tensor.matmul(zb_ps[0:MP, u, :],
                                     lhsT=selbias[:, 2 * i + u, :, :],
                                     rhs=zext)
                r = rpool.tile([128, 2, B], bf16)
                nc.scalar.activation(
                    out=r[0:MP], in_=zb_ps[0:MP], func=Act.Relu, bias=0.0,
                    scale=1.0, accum_out=Sacc[0:MP, i:i + 1])
                m = mpool.tile([128, 2, B], bf16)
                nc.vector.tensor_scalar(
                    m[0:MP], zb_ps[0:MP], EPS, None, Alu.is_gt, Alu.add,
                    accum_out=Cacc[0:MP, i:i + 1])
            # group 20 (single) and the anchor-63 tail share one iteration
            zb_ps = wpsum.tile([128, 2, B], f32)
            nc.tensor.matmul(zb_ps[0:MP, 0, :], lhsT=selbias[:, 20, :, :],
                             rhs=zext)
            nc.tensor.matmul(zb_ps[0:42, 1, :], lhsT=selbias_t, rhs=zext)
            r = rpool.tile([128, 2, B], bf16)
            nc.scalar.activation(
                out=r[0:MP, 0], in_=zb_ps[0:MP, 0], func=Act.Relu, bias=0.0,
                scale=1.0, accum_out=Sacc[0:MP, 10:11])
            nc.scalar.activation(
                out=r[0:42, 1], in_=zb_ps[0:42, 1], func=Act.Relu, bias=0.0,
                scale=1.0, accum_out=Sacc[0:42, 11:12])
            m = mpool.tile([128, 2, B], bf16)
            nc.vector.tensor_scalar(
                m[0:MP, 0], zb_ps[0:MP, 0], EPS, None, Alu.is_gt, Alu.add,
                accum_out=Cacc[0:MP, 10:11])
            nc.vector.tensor_scalar(
                m[0:42, 1], zb_ps[0:42, 1], EPS, None, Alu.is_gt, Alu.add,
                accum_out=Cacc[0:42, 11:12])

            # ---- final reduce ----
            SC = singles.tile([128, 2], f32)
            nc.vector.tensor_reduce(SC[:, 0:1], Sacc, Ax.X, Alu.add)
            nc.vector.tensor_reduce(SC[:, 1:2], Cacc, Ax.X, Alu.add)
            fin_ps = spsum.tile([2, 1], f32, tag="sA")
            nc.tensor.matmul(fin_ps, lhsT=SC, rhs=ones_c)
            fin_sb = singles.tile([2, 1], f32)
            nc.scalar.copy(fin_sb, fin_ps)
            nc.sync.dma_start(
                out=out_d.ap().rearrange("(p o) -> p o", o=1), in_=fin_sb)

    nc.compile()
    return nc


def _build_dense():
    """Dense fallback (no class-size assumption)."""
    import concourse.mybir as mybir
    import concourse.tile as tile
    from concourse import bacc
    from concourse.masks import make_identity

    f32 = mybir.dt.float32
    bf16 = mybir.dt.bfloat16
    Alu = mybir.AluOpType
    Act = mybir.ActivationFunctionType
    Ax = mybir.AxisListType

    nc = bacc.Bacc("TRN2", target_bir_lowering=False, debug=False,
                   num_devices=NCORES)

    imgT_d = nc.dram_tensor("imgT", [D, A], f32, kind="ExternalInput")
    sentT_d = nc.dram_tensor("sentT", [D, B], f32, kind="ExternalInput")
    labf_d = nc.dram_tensor("labf", [B], bf16, kind="ExternalInput")
    labc_d = nc.dram_tensor("labc", [A], f32, kind="ExternalInput")
    out_d = nc.dram_tensor("out", [2], f32, kind="ExternalOutput")
    wdram_d = nc.dram_tensor("wdram", [A, W], bf16)

    with tile.TileContext(nc) as tc:
        with ExitStack() as ctx:
            singles = ctx.enter_context(tc.tile_pool(name="singles", bufs=1))
            rpool = ctx.enter_context(tc.tile_pool(name="rpool", bufs=6))
            mpool = ctx.enter_context(tc.tile_pool(name="mpool", bufs=6))
            spsum = ctx.enter_context(
                tc.tile_pool(name="spsum", bufs=1, space="PSUM"))
            wpsum = ctx.enter_context(
                tc.tile_pool(name="wpsum", bufs=3, space="PSUM"))

            ones_r = singles.tile([1, 128], f32)
            nc.vector.memset(ones_r, 1.0)
            ones_c = singles.tile([128, 1], f32)
            nc.vector.memset(ones_c, 1.0)
            ident = singles.tile([64, 64], f32)
            make_identity(nc, ident)

            imgT = singles.tile([128, KT, A], f32)
            nc.sync.dma_start(
                out=imgT, in_=imgT_d.ap().rearrange("(t p) m -> p t m", p=128))
            sentT = singles.tile([128, KT, B], f32)
            nc.sync.dma_start(
                out=sentT, in_=sentT_d.ap().rearrange("(t p) m -> p t m", p=128))
            lab_row = singles.tile([1, B], f32)
            nc.sync.dma_start(
                out=lab_row, in_=labf_d.ap().rearrange("(o b) -> o b", o=1))
            labc_col = singles.tile([A, 1], f32)
            nc.sync.dma_start(
                out=labc_col, in_=labc_d.ap().rearrange("(a o) -> a o", o=1))

            pw_ps = spsum.tile([A, B], f32)
            for kt in range(KT):
                nc.tensor.matmul(pw_ps, lhsT=imgT[:, kt, :], rhs=sentT[:, kt, :],
                                 start=(kt == 0), stop=(kt == KT - 1))

            labB_ps = spsum.tile([A, B], f32)
            nc.tensor.matmul(labB_ps, lhsT=ones_r[:, :A], rhs=lab_row)
            eqP = singles.tile([A, B], f32)
            nc.vector.tensor_scalar(eqP, labB_ps, labc_col, None, Alu.is_equal)
            penP = singles.tile([A, B], f32)
            nc.vector.tensor_scalar(penP, eqP, 1.0, BIG, Alu.subtract, Alu.mult)
            penN = singles.tile([A, B], f32)
            nc.vector.tensor_scalar(penN, eqP, -BIG, None, Alu.mult)

            w = singles.tile([A, B], f32)
            nc.vector.tensor_scalar(w, pw_ps, MARGIN, None, Alu.add)
            nc.vector.tensor_mul(w, w, eqP)
            nc.vector.tensor_add(w, w, penP)
            negneq = singles.tile([A, B], f32)
            nc.vector.tensor_scalar(negneq, eqP, 1.0, -1.0, Alu.subtract,
                                    Alu.mult)
            z = singles.tile([A, B], f32)
            nc.vector.tensor_scalar(z, pw_ps, -1.0, None, Alu.mult)
            nc.vector.tensor_mul(z, z, negneq)
            nc.vector.tensor_add(z, z, penN)

            zTs = singles.tile([128, NT, A], f32)
            for j in range(NT):
                zt_ps = spsum.tile([128, A], f32)
                nc.tensor.transpose(zt_ps, z[:, j * 128:(j + 1) * 128], ident)
                nc.scalar.copy(zTs[:, j, :], zt_ps)

            Sacc = singles.tile([128, A * NT], f32)
            Cacc = singles.tile([128, A * NT], f32)

            for a in range(A):
                wb_ps = wpsum.tile([128, B], f32)
                nc.tensor.matmul(
                    wb_ps, lhsT=ident[:, a:a + 1].broadcast_to([A, 128]), rhs=w)
                for j in range(NT):
                    col = a * NT + j
                    r = rpool.tile([128, B], bf16)
                    nc.scalar.activation(
                        out=r, in_=wb_ps, func=Act.Relu,
                        bias=zTs[:, j, a:a + 1], scale=1.0,
                        accum_out=Sacc[:, col:col + 1])
                    m = mpool.tile([128, B], bf16)
                    nc.vector.tensor_scalar(
                        m, r, EPS, None, Alu.is_gt, Alu.add,
                        accum_out=Cacc[:, col:col + 1])

            SC = singles.tile([128, 2], f32)
            nc.vector.tensor_reduce(SC[:, 0:1], Sacc, Ax.X, Alu.add)
            nc.vector.tensor_reduce(SC[:, 1:2], Cacc, Ax.X, Alu.add)
            fin_ps = spsum.tile([2, 1], f32)
            nc.tensor.matmul(fin_ps, lhsT=SC, rhs=ones_c)
            fin_sb = singles.tile([2, 1], f32)
            nc.scalar.copy(fin_sb, fin_ps)
            nc.sync.dma_start(
                out=out_d.ap().rearrange("(p o) -> p o", o=1), in_=fin_sb)

    nc.compile()
    return nc


def _get_nc(variant):
    key = f"nc_{variant}"
    if key not in _CACHE:
        _CACHE[key] = _build_win() if variant == "win" else _build_dense()
    return _CACHE[key]


def _selc():
    if "selc" not in _CACHE:
        np_ = A // 2
        s = np.zeros((A, np_, 2, 64), np.float16)
        for ar in range(np_):
            for h in range(2):
                s[ar + 32 * h, ar, h, :] = 1.0
        _CACHE["selc"] = np.ascontiguousarray(s.reshape(A, -1))
    return _CACHE["selc"]


def _prep(labels, image_embeddings, sentence_embeddings):
    """Class-sort the batch; build per-core input maps."""
    labels = np.ascontiguousarray(labels)
    img = np.ascontiguousarray(image_embeddings, dtype=np.float32)
    sent = np.ascontiguousarray(sentence_embeddings, dtype=np.float32)
    counts = np.bincount(labels.astype(np.int64))
    variant = "win" if counts.max() <= MAXC_WIN else "dense"

    perm = np.argsort(labels, kind="stable")
    labs = labels[perm].astype(np.float32)
    imgT = np.ascontiguousarray(img[perm].T)    # [D, B]
    sentT = np.ascontiguousarray(sent[perm].T)  # [D, B]
    if variant == "win":
        imgT = imgT.astype(np.float16)
        sentT = sentT.astype(np.float16)

    maps = []
    for i in range(NCORES):
        c0 = i * A
        m = {"labc": np.ascontiguousarray(labs[c0:c0 + A])}
        if variant == "win":
            # rotate the sentence axis so this core's 128-wide window
            # [c0-32, c0+96) lands at columns [0, W)
            rot = (np.arange(B) + c0 - 32) % B
            m["packT"] = np.ascontiguousarray(
                np.concatenate([imgT[:, c0:c0 + A], sentT[:, rot]], axis=1))
            m["labf"] = np.ascontiguousarray(labs[rot]).astype(np.float16)
        else:
            m["imgT"] = np.ascontiguousarray(imgT[:, c0:c0 + A])
            m["sentT"] = sentT
            m["labf"] = labs
        maps.append(m)
    return variant, maps


def run_all(labels, image_embeddings, sentence_embeddings, trace=False):
    from concourse.bass_utils import run_bass_kernel_spmd
    variant, maps = _prep(labels, image_embeddings, sentence_embeddings)
    nc = _get_nc(variant)
    res = run_bass_kernel_spmd(nc, maps, list(range(NCORES)), trace=trace)
    parts = np.stack([res.results[i]["out"] for i in range(NCORES)])
    s = float(parts[:, 0].sum())
    c = float(parts[:, 1].sum())
    loss = np.float32(s / (c + EPS))
    return np.asarray(loss, dtype=np.float32), res


def kernel(labels, image_embeddings, sentence_embeddings):
    out, _ = run_all(labels, image_embeddings, sentence_embeddings)
    return out



# revision 5
# speedup vs baseline: 1.7649x; 1.7649x over previous
"""Trainium2 Bass kernel for BatchAll triplet loss.

Reference computation (B=512, D=1024):
    pw = img @ sent.T                                  [B, B]
    t[a,p,n] = pw[a,p] - pw[a,n] + margin
    valid[a,p,n] = (lab[a]==lab[p]) & (lab[a]!=lab[n])
    loss = sum(relu(valid*t)) / (count(valid*t > EPS) + EPS)

Strategy: the batch is class-sorted on the host (a pure permutation of the
(image, sentence, label) triples; the loss is permutation invariant), then
anchors are sharded across 8 cores (64 each, C = core*64). After sorting,
all positives of anchor a live in a contiguous class run inside the core's
128-wide sentence window [C-32, C+96) (holds when max class size <= 33;
dense fallback otherwise). Each core enumerates its actual valid (a,p)
pairs (sum of class sizes over its anchors, ~320 for uniform labels) and
packs them onto partitions: tiles of 128 pairs, free axis = all 512 n.

Per core, with the sentence axis pre-rotated so the window is cols [0,128):
    pw[a,n] (PE, 8 k-tiles)                              [64, 512]
    zext[a,n] = -pw + penM  (penM = -30000 on same-label n)  fp16
    per pair-tile t (T = ceil(maxpairs/128)):
        Z[k,n] = zext[a_k, n]        (PE one-hot broadcast)  [128, 512]
        w[k]   = pw[a_k,p_k]+margin  (DVE: Z's window cols already hold
                 -pw[a_k,p_k]-30000 at col j_k; one-hot dot + init scalar)
        ACT  relu(Z + w) with accum_out row-sums -> Sacc
        DVE  count r > EPS with accum_out        -> Cacc
Host combines the 8 (sum, count) pairs and divides.
"""

import numpy as np
from contextlib import ExitStack

B = 512
D = 1024
NCORES = 8
A = B // NCORES   # 64 anchors per core
KT = D // 128     # 8 contraction tiles
NT = B // 128     # 4 n-tiles per anchor (dense variant)
W = 128           # per-core sentence window width
MARGIN = 0.2
EPS = 1e-16
BIG = 1e30
BIGW = 30000.0
MAXC_WIN = 33     # pair variant valid iff max class size <= this

_CACHE = {}


def _build_pairs(T):
    """Pair-packed kernel: T tiles of 128 (anchor, positive) pairs."""
    import concourse.mybir as mybir
    import concourse.tile as tile
    from concourse import bacc

    f32 = mybir.dt.float32
    f16 = mybir.dt.float16
    Alu = mybir.AluOpType
    Act = mybir.ActivationFunctionType
    Ax = mybir.AxisListType

    nc = bacc.Bacc("TRN2", target_bir_lowering=False, debug=False,
                   num_devices=NCORES)

    K = T * 128
    packT_d = nc.dram_tensor("packT", [D, A + B], f16, kind="ExternalInput")
    penM_d = nc.dram_tensor("penM", [A, B], f16, kind="ExternalInput")
    selA_d = nc.dram_tensor("selA", [A + 1, K], f16, kind="ExternalInput")
    oneJ_d = nc.dram_tensor("oneJ", [128, T, W], f16, kind="ExternalInput")
    out_d = nc.dram_tensor("out", [2], f32, kind="ExternalOutput")

    with tile.TileContext(nc) as tc:
        with ExitStack() as ctx:
            singles = ctx.enter_context(tc.tile_pool(name="singles", bufs=1))
            rpool = ctx.enter_context(tc.tile_pool(name="rpool", bufs=3))
            mpool = ctx.enter_context(tc.tile_pool(name="mpool", bufs=3))
            jpool = ctx.enter_context(tc.tile_pool(name="jpool", bufs=2))
            spsum = ctx.enter_context(
                tc.tile_pool(name="spsum", bufs=1, space="PSUM"))
            wpsum = ctx.enter_context(
                tc.tile_pool(name="wpsum", bufs=3, space="PSUM"))
            gpsum = ctx.enter_context(
                tc.tile_pool(name="gpsum", bufs=2, space="PSUM"))

            # ---- small inputs first (gpsimd queue), packT split per
            # k-tile over two queues so PE starts early ----
            selA = singles.tile([A + 1, K], f16)
            nc.gpsimd.dma_start(out=selA, in_=selA_d.ap())
            oneJ = singles.tile([128, T, W], f16)
            nc.gpsimd.dma_start(out=oneJ, in_=oneJ_d.ap())
            penM = singles.tile([A, B], f16)
            nc.gpsimd.dma_start(out=penM, in_=penM_d.ap())

            packT = singles.tile([128, KT, A + B], f16)
            packT_v = packT_d.ap().rearrange("(t p) m -> p t m", p=128)
            for kt in range(KT):
                eng = nc.sync if kt % 2 == 0 else nc.scalar
                eng.dma_start(out=packT[:, kt, :], in_=packT_v[:, kt, :])
            imgT = packT[:, :, 0:A]
            sentT = packT[:, :, A:A + B]

            # ---- constants / accumulators ----
            ones_c = singles.tile([128, 1], f32)
            nc.vector.memset(ones_c, 1.0)
            Sacc = singles.tile([128, T], f32)
            nc.vector.memset(Sacc, 0.0)
            Cacc = singles.tile([128, T], f32)
            nc.vector.memset(Cacc, 0.0)
            wcol = singles.tile([128, T], f32)

            # ---- pairwise rows (sentT pre-rotated: window = cols 0..W) ----
            pw_ps = spsum.tile([A, B], f32, tag="sA")
            for kt in range(KT):
                nc.tensor.matmul(pw_ps, lhsT=imgT[:, kt, :], rhs=sentT[:, kt, :],
                                 start=(kt == 0), stop=(kt == KT - 1))

            # ---- z rows: zext[a,n] = -pw[a,n] + penM[a,n]; row 64 = -BIGW
            # (selected by padded pairs so their t stays negative) ----
            zext = singles.tile([A + 1, B], f16)
            nc.vector.memset(zext[A:A + 1, :], -BIGW)
            nc.vector.scalar_tensor_tensor(zext[0:A, :], pw_ps, -1.0, penM,
                                           Alu.mult, Alu.add)
            # clean fp16 copy of the pw window for the wcol gather (the
            # masked zext holds -pw-30000 whose fp16 ulp is 16)
            pwin = singles.tile([A, W], f16)
            nc.scalar.copy(pwin, pw_ps[:, 0:W])

            # ---- main loop: one tile of 128 pairs per iteration ----
            for t in range(T):
                sl = selA[:, t * 128:(t + 1) * 128]
                g_ps = gpsum.tile([128, W], f32)
                nc.tensor.matmul(g_ps, lhsT=sl[0:A, :], rhs=pwin)
                z_ps = wpsum.tile([128, B], f32)
                nc.tensor.matmul(z_ps, lhsT=sl, rhs=zext)
                # wcol[k] = margin + pw[a_k, p_k]  (tensor_tensor_reduce
                # would fuse these but fails on HW; split into proven ops)
                junk = jpool.tile([128, W], f32)
                nc.vector.tensor_mul(junk, g_ps, oneJ[:, t, :])
                wr = jpool.tile([128, 1], f32)
                nc.vector.tensor_reduce(wr, junk, Ax.X, Alu.add)
                nc.vector.tensor_scalar(wcol[:, t:t + 1], wr, MARGIN, None,
                                        Alu.add)
                r = rpool.tile([128, B], f16)
                nc.scalar.activation(
                    out=r, in_=z_ps, func=Act.Relu,
                    bias=wcol[:, t:t + 1], scale=1.0,
                    accum_out=Sacc[:, t:t + 1])
                m = mpool.tile([128, B], f16)
                nc.vector.tensor_scalar(
                    m, r, EPS, None, Alu.is_gt, Alu.add,
                    accum_out=Cacc[:, t:t + 1])

            # ---- final reduce ----
            SC = singles.tile([128, 2], f32)
            nc.vector.tensor_reduce(SC[:, 0:1], Sacc, Ax.X, Alu.add)
            nc.vector.tensor_reduce(SC[:, 1:2], Cacc, Ax.X, Alu.add)
            fin_ps = spsum.tile([2, 1], f32, tag="sA")
            nc.tensor.matmul(fin_ps, lhsT=SC, rhs=ones_c)
            fin_sb = singles.tile([2, 1], f32)
            nc.scalar.copy(fin_sb, fin_ps)
            nc.sync.dma_start(
                out=out_d.ap().rearrange("(p o) -> p o", o=1), in_=fin_sb)

    nc.compile()
    return nc


def _build_dense():
    """Dense fallback (no class-size assumption)."""
    import concourse.mybir as mybir
    import concourse.tile as tile
    from concourse import bacc
    from concourse.masks import make_identity

    f32 = mybir.dt.float32
    bf16 = mybir.dt.bfloat16
    Alu = mybir.AluOpType
    Act = mybir.ActivationFunctionType
    Ax = mybir.AxisListType

    nc = bacc.Bacc("TRN2", target_bir_lowering=False, debug=False,
                   num_devices=NCORES)

    imgT_d = nc.dram_tensor("imgT", [D, A], f32, kind="ExternalInput")
    sentT_d = nc.dram_tensor("sentT", [D, B], f32, kind="ExternalInput")
    labf_d = nc.dram_tensor("labf", [B], bf16, kind="ExternalInput")
    labc_d = nc.dram_tensor("labc", [A], f32, kind="ExternalInput")
    out_d = nc.dram_tensor("out", [2], f32, kind="ExternalOutput")

    with tile.TileContext(nc) as tc:
        with ExitStack() as ctx:
            singles = ctx.enter_context(tc.tile_pool(name="singles", bufs=1))
            rpool = ctx.enter_context(tc.tile_pool(name="rpool", bufs=6))
            mpool = ctx.enter_context(tc.tile_pool(name="mpool", bufs=6))
            spsum = ctx.enter_context(
                tc.tile_pool(name="spsum", bufs=1, space="PSUM"))
            wpsum = ctx.enter_context(
                tc.tile_pool(name="wpsum", bufs=3, space="PSUM"))

            ones_r = singles.tile([1, 128], f32)
            nc.vector.memset(ones_r, 1.0)
            ones_c = singles.tile([128, 1], f32)
            nc.vector.memset(ones_c, 1.0)
            ident = singles.tile([64, 64], f32)
            make_identity(nc, ident)

            imgT = singles.tile([128, KT, A], f32)
            nc.sync.dma_start(
                out=imgT, in_=imgT_d.ap().rearrange("(t p) m -> p t m", p=128))
            sentT = singles.tile([128, KT, B], f32)
            nc.sync.dma_start(
                out=sentT, in_=sentT_d.ap().rearrange("(t p) m -> p t m", p=128))
            lab_row = singles.tile([1, B], f32)
            nc.sync.dma_start(
                out=lab_row, in_=labf_d.ap().rearrange("(o b) -> o b", o=1))
            labc_col = singles.tile([A, 1], f32)
            nc.sync.dma_start(
                out=labc_col, in_=labc_d.ap().rearrange("(a o) -> a o", o=1))

            pw_ps = spsum.tile([A, B], f32)
            for kt in range(KT):
                nc.tensor.matmul(pw_ps, lhsT=imgT[:, kt, :], rhs=sentT[:, kt, :],
                                 start=(kt == 0), stop=(kt == KT - 1))

            labB_ps = spsum.tile([A, B], f32)
            nc.tensor.matmul(labB_ps, lhsT=ones_r[:, :A], rhs=lab_row)
            eqP = singles.tile([A, B], f32)
            nc.vector.tensor_scalar(eqP, labB_ps, labc_col, None, Alu.is_equal)
            penP = singles.tile([A, B], f32)
            nc.vector.tensor_scalar(penP, eqP, 1.0, BIG, Alu.subtract, Alu.mult)
            penN = singles.tile([A, B], f32)
            nc.vector.tensor_scalar(penN, eqP, -BIG, None, Alu.mult)

            w = singles.tile([A, B], f32)
            nc.vector.tensor_scalar(w, pw_ps, MARGIN, None, Alu.add)
            nc.vector.tensor_mul(w, w, eqP)
            nc.vector.tensor_add(w, w, penP)
            negneq = singles.tile([A, B], f32)
            nc.vector.tensor_scalar(negneq, eqP, 1.0, -1.0, Alu.subtract,
                                    Alu.mult)
            z = singles.tile([A, B], f32)
            nc.vector.tensor_scalar(z, pw_ps, -1.0, None, Alu.mult)
            nc.vector.tensor_mul(z, z, negneq)
            nc.vector.tensor_add(z, z, penN)

            zTs = singles.tile([128, NT, A], f32)
            for j in range(NT):
                zt_ps = spsum.tile([128, A], f32)
                nc.tensor.transpose(zt_ps, z[:, j * 128:(j + 1) * 128], ident)
                nc.scalar.copy(zTs[:, j, :], zt_ps)

            Sacc = singles.tile([128, A * NT], f32)
            Cacc = singles.tile([128, A * NT], f32)

            for a in range(A):
                wb_ps = wpsum.tile([128, B], f32)
                nc.tensor.matmul(
                    wb_ps, lhsT=ident[:, a:a + 1].broadcast_to([A, 128]), rhs=w)
                for j in range(NT):
                    col = a * NT + j
                    r = rpool.tile([128, B], bf16)
                    nc.scalar.activation(
                        out=r, in_=wb_ps, func=Act.Relu,
                        bias=zTs[:, j, a:a + 1], scale=1.0,
                        accum_out=Sacc[:, col:col + 1])
                    m = mpool.tile([128, B], bf16)
                    nc.vector.tensor_scalar(
                        m, r, EPS, None, Alu.is_gt, Alu.add,
                        accum_out=Cacc[:, col:col + 1])

            SC = singles.tile([128, 2], f32)
            nc.vector.tensor_reduce(SC[:, 0:1], Sacc, Ax.X, Alu.add)
            nc.vector.tensor_reduce(SC[:, 1:2], Cacc, Ax.X, Alu.add)
            fin_ps = spsum.tile([2, 1], f32)
            nc.tensor.matmul(fin_ps, lhsT=SC, rhs=ones_c)
            fin_sb = singles.tile([2, 1], f32)
            nc.scalar.copy(fin_sb, fin_ps)
            nc.sync.dma_start(
                out=out_d.ap().rearrange("(p o) -> p o", o=1), in_=fin_sb)

    nc.compile()
    return nc


def _get_nc(variant, T=0):
    key = f"nc_{variant}_{T}"
    if key not in _CACHE:
        _CACHE[key] = (_build_pairs(T) if variant == "pairs"
                       else _build_dense())
    return _CACHE[key]


def _prep(labels, image_embeddings, sentence_embeddings):
    """Class-sort the batch; build per-core input maps."""
    labels = np.ascontiguousarray(labels).astype(np.int64)
    img = np.ascontiguousarray(image_embeddings, dtype=np.float32)
    sent = np.ascontiguousarray(sentence_embeddings, dtype=np.float32)
    counts = np.bincount(labels, minlength=1)
    maxc = counts.max()

    perm = np.argsort(labels, kind="stable")
    labs = labels[perm]

    if maxc > MAXC_WIN:
        imgT = np.ascontiguousarray(img[perm].T)    # [D, B]
        sentT = np.ascontiguousarray(sent[perm].T)  # [D, B]
        labsf = labs.astype(np.float32)
        maps = []
        for i in range(NCORES):
            c0 = i * A
            maps.append({
                "imgT": np.ascontiguousarray(imgT[:, c0:c0 + A]),
                "sentT": sentT,
                "labf": labsf,
                "labc": np.ascontiguousarray(labsf[c0:c0 + A]),
            })
        return "dense", 0, maps

    imgT = np.ascontiguousarray(img[perm].T).astype(np.float16)
    sentT = np.ascontiguousarray(sent[perm].T).astype(np.float16)

    # class run start/size per sorted position
    starts = np.concatenate([[0], np.cumsum(counts)])
    s_a = starts[labs]            # run start of each anchor
    n_a = counts[labs]            # run length of each anchor
    maxK = max(int(n_a[c0:c0 + A].sum()) for c0 in range(0, B, A))
    T = (maxK + 127) // 128

    maps = []
    for i in range(NCORES):
        c0 = i * A
        rot = (np.arange(B) + c0 - 32) % B
        packT = np.ascontiguousarray(
            np.concatenate([imgT[:, c0:c0 + A], sentT[:, rot]], axis=1))
        # penM[a, n] = -BIGW where rotated label n matches anchor label
        eq = labs[rot][None, :] == labs[c0:c0 + A][:, None]
        penM = np.where(eq, np.float16(-BIGW), np.float16(0.0))
        # pair list: for each local anchor a, all p in its class run
        K = T * 128
        selA = np.zeros((A + 1, K), np.float16)
        oneJ = np.zeros((128, T, W), np.float16)
        k = 0
        for a in range(A):
            ga = c0 + a
            for p in range(int(s_a[ga]), int(s_a[ga] + n_a[ga])):
                j = p - (c0 - 32)
                selA[a, k] = 1.0
                oneJ[k % 128, k // 128, j] = 1.0
                k += 1
        selA[A, k:] = 1.0   # pads select zext row 64 (zeros)
        maps.append({
            "packT": packT,
            "penM": np.ascontiguousarray(penM),
            "selA": selA,
            "oneJ": oneJ,
        })
    return "pairs", T, maps


def run_all(labels, image_embeddings, sentence_embeddings, trace=False):
    from concourse.bass_utils import run_bass_kernel_spmd
    variant, T, maps = _prep(labels, image_embeddings, sentence_embeddings)
    nc = _get_nc(variant, T)
    res = run_bass_kernel_spmd(nc, maps, list(range(NCORES)), trace=trace)
    parts = np.stack([res.results[i]["out"] for i in range(NCORES)])
    s = float(parts[:, 0].sum())
    c = float(parts[:, 1].sum())
    loss = np.float32(s / (c + EPS))
    return np.asarray(loss, dtype=np.float32), res


def kernel(labels, image_embeddings, sentence_embeddings):
    out, _ = run_all(labels, image_embeddings, sentence_embeddings)
    return out


# revision 10
# speedup vs baseline: 1.9140x; 1.0845x over previous
"""Trainium2 Bass kernel for BatchAll triplet loss.

Reference computation (B=512, D=1024):
    pw = img @ sent.T                                  [B, B]
    t[a,p,n] = pw[a,p] - pw[a,n] + margin
    valid[a,p,n] = (lab[a]==lab[p]) & (lab[a]!=lab[n])
    loss = sum(relu(valid*t)) / (count(valid*t > EPS) + EPS)

Strategy: the batch is class-sorted on the host (a pure permutation of the
(image, sentence, label) triples; the loss is permutation invariant), then
anchors are sharded across 8 cores (64 each, C = core*64). After sorting,
all positives of anchor a live in a contiguous class run inside the core's
128-wide sentence window [C-32, C+96) (holds when max class size <= 33;
dense fallback otherwise). Each core enumerates its actual valid (a,p)
pairs (sum of class sizes over its anchors, ~320 for uniform labels) and
packs them onto partitions: tiles of 128 pairs, free axis = all 512 n.

Per core, with the sentence axis pre-rotated so the window is cols [0,128):
    pw[a,n] (PE, 8 k-tiles)                              [64, 512]
    zext[a,n] = -pw + penM  (penM = -30000 on same-label n)  fp16
    per pair-tile t (T = ceil(maxpairs/128)):
        Z[k,n] = zext[a_k, n]        (PE one-hot broadcast)  [128, 512]
        w[k]   = pw[a_k,p_k]+margin  (DVE: Z's window cols already hold
                 -pw[a_k,p_k]-30000 at col j_k; one-hot dot + init scalar)
        ACT  relu(Z + w) with accum_out row-sums -> Sacc
        DVE  count r > EPS with accum_out        -> Cacc
Host combines the 8 (sum, count) pairs and divides.
"""

import numpy as np
from contextlib import ExitStack

B = 512
D = 1024
NCORES = 8
A = B // NCORES   # 64 anchors per core
KT = D // 128     # 8 contraction tiles
NT = B // 128     # 4 n-tiles per anchor (dense variant)
W = 128           # per-core sentence window width
MARGIN = 0.2
EPS = 1e-16
BIG = 1e30
BIGW = 30000.0
MAXC_WIN = 33     # pair variant valid iff max class size <= this

_CACHE = {}


def _build_pairs(T):
    """Pair-packed kernel: T tiles of 128 (anchor, positive) pairs."""
    import concourse.mybir as mybir
    import concourse.tile as tile
    from concourse import bacc

    f32 = mybir.dt.float32
    f16 = mybir.dt.float16
    Alu = mybir.AluOpType
    Act = mybir.ActivationFunctionType
    Ax = mybir.AxisListType

    nc = bacc.Bacc("TRN2", target_bir_lowering=False, debug=False,
                   num_devices=NCORES)

    K = T * 128
    packT_d = nc.dram_tensor("packT", [D, A + B], f16, kind="ExternalInput")
    penM_d = nc.dram_tensor("penM", [A, B], f16, kind="ExternalInput")
    selA_d = nc.dram_tensor("selA", [A + 1, K], f16, kind="ExternalInput")
    oneJ_d = nc.dram_tensor("oneJ", [128, T, W], f16, kind="ExternalInput")
    out_d = nc.dram_tensor("out", [2], f32, kind="ExternalOutput")

    with tile.TileContext(nc) as tc:
        with ExitStack() as ctx:
            singles = ctx.enter_context(tc.tile_pool(name="singles", bufs=1))
            rpool = ctx.enter_context(tc.tile_pool(name="rpool", bufs=3))
            mpool = ctx.enter_context(tc.tile_pool(name="mpool", bufs=3))
            jpool = ctx.enter_context(tc.tile_pool(name="jpool", bufs=2))
            spsum = ctx.enter_context(
                tc.tile_pool(name="spsum", bufs=1, space="PSUM"))
            wpsum = ctx.enter_context(
                tc.tile_pool(name="wpsum", bufs=3, space="PSUM"))
            gpsum = ctx.enter_context(
                tc.tile_pool(name="gpsum", bufs=2, space="PSUM"))

            # ---- small inputs first (gpsimd queue), packT split per
            # k-tile over two queues so PE starts early ----
            penM = singles.tile([A, B], f16)
            nc.gpsimd.dma_start(out=penM, in_=penM_d.ap())

            packT = singles.tile([128, KT, A + B], f16)
            packT_v = packT_d.ap().rearrange("(t p) m -> p t m", p=128)
            qs = [nc.sync, nc.scalar, nc.gpsimd]
            for kt in range(KT):
                qs[kt % 3].dma_start(out=packT[:, kt, :], in_=packT_v[:, kt, :])
            imgT = packT[:, :, 0:A]
            sentT = packT[:, :, A:A + B]

            selA = singles.tile([A + 1, K], f16)
            nc.gpsimd.dma_start(out=selA, in_=selA_d.ap())
            oneJ = singles.tile([128, T, W], f16)
            nc.gpsimd.dma_start(out=oneJ, in_=oneJ_d.ap())

            # ---- constants / accumulators ----
            ones_c = singles.tile([128, 1], f32)
            nc.vector.memset(ones_c, 1.0)
            Sacc = singles.tile([128, T], f32)
            nc.vector.memset(Sacc, 0.0)
            Cacc = singles.tile([128, T], f32)
            nc.vector.memset(Cacc, 0.0)
            wcol = singles.tile([128, T], f32)

            # ---- pairwise rows (sentT pre-rotated: window = cols 0..W) ----
            pw_ps = spsum.tile([A, B], f32, tag="sA")
            for kt in range(KT):
                nc.tensor.matmul(pw_ps, lhsT=imgT[:, kt, :], rhs=sentT[:, kt, :],
                                 start=(kt == 0), stop=(kt == KT - 1))

            # clean fp16 copy of the pw window for the wcol gather (the
            # masked zext holds -pw-30000 whose fp16 ulp is 16); on the
            # vector engine so it lands before zext and the g matmuls can
            # beat the z matmuls onto the PE
            pwin = singles.tile([A, W], f16)
            nc.vector.tensor_scalar(pwin, pw_ps[:, 0:W], 0.0, None, Alu.add)

            # ---- z rows: zext[a,n] = -pw[a,n] + penM[a,n]; row 64 = -BIGW
            # (selected by padded pairs so their t stays negative) ----
            zext = singles.tile([A + 1, B], f16)
            nc.vector.memset(zext[A:A + 1, :], -BIGW)
            nc.vector.scalar_tensor_tensor(zext[0:A, :], pw_ps, -1.0, penM,
                                           Alu.mult, Alu.add)

            # ---- wcol gathers (g matmuls emitted before the z matmuls) ----
            for t in range(T):
                sl = selA[:, t * 128:(t + 1) * 128]
                g_ps = gpsum.tile([128, W], f32)
                nc.tensor.matmul(g_ps, lhsT=sl[0:A, :], rhs=pwin)
                # wcol[k] = margin + pw[a_k, p_k]  (tensor_tensor_reduce
                # would fuse these but fails on HW; split into proven ops)
                junk = jpool.tile([128, W], f32)
                nc.vector.tensor_mul(junk, g_ps, oneJ[:, t, :])
                wr = jpool.tile([128, 1], f32)
                nc.vector.tensor_reduce(wr, junk, Ax.X, Alu.add)
                nc.vector.tensor_scalar(wcol[:, t:t + 1], wr, MARGIN, None,
                                        Alu.add)

            # ---- main loop: one tile of 128 pairs per iteration ----
            for t in range(T):
                sl = selA[:, t * 128:(t + 1) * 128]
                z_ps = wpsum.tile([128, B], f32)
                nc.tensor.matmul(z_ps, lhsT=sl, rhs=zext)
                r = rpool.tile([128, B], f16)
                nc.scalar.activation(
                    out=r, in_=z_ps, func=Act.Relu,
                    bias=wcol[:, t:t + 1], scale=1.0,
                    accum_out=Sacc[:, t:t + 1])
                m = mpool.tile([128, B], f16)
                nc.vector.tensor_scalar(
                    m, r, EPS, None, Alu.is_gt, Alu.add,
                    accum_out=Cacc[:, t:t + 1])

            # ---- final reduce ----
            SC = singles.tile([128, 2], f32)
            nc.vector.tensor_reduce(SC[:, 0:1], Sacc, Ax.X, Alu.add)
            nc.vector.tensor_reduce(SC[:, 1:2], Cacc, Ax.X, Alu.add)
            fin_ps = spsum.tile([2, 1], f32, tag="sA")
            nc.tensor.matmul(fin_ps, lhsT=SC, rhs=ones_c)
            fin_sb = singles.tile([2, 1], f32)
            nc.scalar.copy(fin_sb, fin_ps)
            nc.sync.dma_start(
                out=out_d.ap().rearrange("(p o) -> p o", o=1), in_=fin_sb)

    nc.compile()
    return nc


def _build_dense():
    """Dense fallback (no class-size assumption)."""
    import concourse.mybir as mybir
    import concourse.tile as tile
    from concourse import bacc
    from concourse.masks import make_identity

    f32 = mybir.dt.float32
    bf16 = mybir.dt.bfloat16
    Alu = mybir.AluOpType
    Act = mybir.ActivationFunctionType
    Ax = mybir.AxisListType

    nc = bacc.Bacc("TRN2", target_bir_lowering=False, debug=False,
                   num_devices=NCORES)

    imgT_d = nc.dram_tensor("imgT", [D, A], f32, kind="ExternalInput")
    sentT_d = nc.dram_tensor("sentT", [D, B], f32, kind="ExternalInput")
    labf_d = nc.dram_tensor("labf", [B], bf16, kind="ExternalInput")
    labc_d = nc.dram_tensor("labc", [A], f32, kind="ExternalInput")
    out_d = nc.dram_tensor("out", [2], f32, kind="ExternalOutput")

    with tile.TileContext(nc) as tc:
        with ExitStack() as ctx:
            singles = ctx.enter_context(tc.tile_pool(name="singles", bufs=1))
            rpool = ctx.enter_context(tc.tile_pool(name="rpool", bufs=6))
            mpool = ctx.enter_context(tc.tile_pool(name="mpool", bufs=6))
            spsum = ctx.enter_context(
                tc.tile_pool(name="spsum", bufs=1, space="PSUM"))
            wpsum = ctx.enter_context(
                tc.tile_pool(name="wpsum", bufs=3, space="PSUM"))

            ones_r = singles.tile([1, 128], f32)
            nc.vector.memset(ones_r, 1.0)
            ones_c = singles.tile([128, 1], f32)
            nc.vector.memset(ones_c, 1.0)
            ident = singles.tile([64, 64], f32)
            make_identity(nc, ident)

            imgT = singles.tile([128, KT, A], f32)
            nc.sync.dma_start(
                out=imgT, in_=imgT_d.ap().rearrange("(t p) m -> p t m", p=128))
            sentT = singles.tile([128, KT, B], f32)
            nc.sync.dma_start(
                out=sentT, in_=sentT_d.ap().rearrange("(t p) m -> p t m", p=128))
            lab_row = singles.tile([1, B], f32)
            nc.sync.dma_start(
                out=lab_row, in_=labf_d.ap().rearrange("(o b) -> o b", o=1))
            labc_col = singles.tile([A, 1], f32)
            nc.sync.dma_start(
                out=labc_col, in_=labc_d.ap().rearrange("(a o) -> a o", o=1))

            pw_ps = spsum.tile([A, B], f32)
            for kt in range(KT):
                nc.tensor.matmul(pw_ps, lhsT=imgT[:, kt, :], rhs=sentT[:, kt, :],
                                 start=(kt == 0), stop=(kt == KT - 1))

            labB_ps = spsum.tile([A, B], f32)
            nc.tensor.matmul(labB_ps, lhsT=ones_r[:, :A], rhs=lab_row)
            eqP = singles.tile([A, B], f32)
            nc.vector.tensor_scalar(eqP, labB_ps, labc_col, None, Alu.is_equal)
            penP = singles.tile([A, B], f32)
            nc.vector.tensor_scalar(penP, eqP, 1.0, BIG, Alu.subtract, Alu.mult)
            penN = singles.tile([A, B], f32)
            nc.vector.tensor_scalar(penN, eqP, -BIG, None, Alu.mult)

            w = singles.tile([A, B], f32)
            nc.vector.tensor_scalar(w, pw_ps, MARGIN, None, Alu.add)
            nc.vector.tensor_mul(w, w, eqP)
            nc.vector.tensor_add(w, w, penP)
            negneq = singles.tile([A, B], f32)
            nc.vector.tensor_scalar(negneq, eqP, 1.0, -1.0, Alu.subtract,
                                    Alu.mult)
            z = singles.tile([A, B], f32)
            nc.vector.tensor_scalar(z, pw_ps, -1.0, None, Alu.mult)
            nc.vector.tensor_mul(z, z, negneq)
            nc.vector.tensor_add(z, z, penN)

            zTs = singles.tile([128, NT, A], f32)
            for j in range(NT):
                zt_ps = spsum.tile([128, A], f32)
                nc.tensor.transpose(zt_ps, z[:, j * 128:(j + 1) * 128], ident)
                nc.scalar.copy(zTs[:, j, :], zt_ps)

            Sacc = singles.tile([128, A * NT], f32)
            Cacc = singles.tile([128, A * NT], f32)

            for a in range(A):
                wb_ps = wpsum.tile([128, B], f32)
                nc.tensor.matmul(
                    wb_ps, lhsT=ident[:, a:a + 1].broadcast_to([A, 128]), rhs=w)
                for j in range(NT):
                    col = a * NT + j
                    r = rpool.tile([128, B], bf16)
                    nc.scalar.activation(
                        out=r, in_=wb_ps, func=Act.Relu,
                        bias=zTs[:, j, a:a + 1], scale=1.0,
                        accum_out=Sacc[:, col:col + 1])
                    m = mpool.tile([128, B], bf16)
                    nc.vector.tensor_scalar(
                        m, r, EPS, None, Alu.is_gt, Alu.add,
                        accum_out=Cacc[:, col:col + 1])

            SC = singles.tile([128, 2], f32)
            nc.vector.tensor_reduce(SC[:, 0:1], Sacc, Ax.X, Alu.add)
            nc.vector.tensor_reduce(SC[:, 1:2], Cacc, Ax.X, Alu.add)
            fin_ps = spsum.tile([2, 1], f32)
            nc.tensor.matmul(fin_ps, lhsT=SC, rhs=ones_c)
            fin_sb = singles.tile([2, 1], f32)
            nc.scalar.copy(fin_sb, fin_ps)
            nc.sync.dma_start(
                out=out_d.ap().rearrange("(p o) -> p o", o=1), in_=fin_sb)

    nc.compile()
    return nc


def _get_nc(variant, T=0):
    key = f"nc_{variant}_{T}"
    if key not in _CACHE:
        _CACHE[key] = (_build_pairs(T) if variant == "pairs"
                       else _build_dense())
    return _CACHE[key]


def _prep(labels, image_embeddings, sentence_embeddings):
    """Class-sort the batch; build per-core input maps."""
    labels = np.ascontiguousarray(labels).astype(np.int64)
    img = np.ascontiguousarray(image_embeddings, dtype=np.float32)
    sent = np.ascontiguousarray(sentence_embeddings, dtype=np.float32)
    counts = np.bincount(labels, minlength=1)
    maxc = counts.max()

    perm = np.argsort(labels, kind="stable")
    labs = labels[perm]

    if maxc > MAXC_WIN:
        imgT = np.ascontiguousarray(img[perm].T)    # [D, B]
        sentT = np.ascontiguousarray(sent[perm].T)  # [D, B]
        labsf = labs.astype(np.float32)
        maps = []
        for i in range(NCORES):
            c0 = i * A
            maps.append({
                "imgT": np.ascontiguousarray(imgT[:, c0:c0 + A]),
                "sentT": sentT,
                "labf": labsf,
                "labc": np.ascontiguousarray(labsf[c0:c0 + A]),
            })
        return "dense", 0, maps

    imgT = np.ascontiguousarray(img[perm].T).astype(np.float16)
    sentT = np.ascontiguousarray(sent[perm].T).astype(np.float16)

    # class run start/size per sorted position
    starts = np.concatenate([[0], np.cumsum(counts)])
    s_a = starts[labs]            # run start of each anchor
    n_a = counts[labs]            # run length of each anchor
    maxK = max(int(n_a[c0:c0 + A].sum()) for c0 in range(0, B, A))
    T = (maxK + 127) // 128

    maps = []
    for i in range(NCORES):
        c0 = i * A
        rot = (np.arange(B) + c0 - 32) % B
        packT = np.ascontiguousarray(
            np.concatenate([imgT[:, c0:c0 + A], sentT[:, rot]], axis=1))
        # penM[a, n] = -BIGW where rotated label n matches anchor label
        eq = labs[rot][None, :] == labs[c0:c0 + A][:, None]
        penM = np.where(eq, np.float16(-BIGW), np.float16(0.0))
        # pair list: for each local anchor a, all p in its class run
        K = T * 128
        selA = np.zeros((A + 1, K), np.float16)
        oneJ = np.zeros((128, T, W), np.float16)
        k = 0
        for a in range(A):
            ga = c0 + a
            for p in range(int(s_a[ga]), int(s_a[ga] + n_a[ga])):
                j = p - (c0 - 32)
                selA[a, k] = 1.0
                oneJ[k % 128, k // 128, j] = 1.0
                k += 1
        selA[A, k:] = 1.0   # pads select zext row 64 (zeros)
        maps.append({
            "packT": packT,
            "penM": np.ascontiguousarray(penM),
            "selA": selA,
            "oneJ": oneJ,
        })
    return "pairs", T, maps


def run_all(labels, image_embeddings, sentence_embeddings, trace=False):
    from concourse.bass_utils import run_bass_kernel_spmd
    variant, T, maps = _prep(labels, image_embeddings, sentence_embeddings)
    nc = _get_nc(variant, T)
    res = run_bass_kernel_spmd(nc, maps, list(range(NCORES)), trace=trace)
    parts = np.stack([res.results[i]["out"] for i in range(NCORES)])
    s = float(parts[:, 0].sum())
    c = float(parts[:, 1].sum())
    loss = np.float32(s / (c + EPS))
    return np.asarray(loss, dtype=np.float32), res


def kernel(labels, image_embeddings, sentence_embeddings):
    out, _ = run_all(labels, image_embeddings, sentence_embeddings)
    return out
